# revision 1
# baseline (speedup 1.0000x reference)
"""Trainium2 Bass kernel: MoE top-k router (top-8 of 64 experts + softmax).

Contract: kernel(logits, top_k) takes the FULL inputs (logits [1048576, 64]
f32, top_k == 8) and returns (topk_idx int64 [N, 8], topk_w f32 [N, 8]),
matching jax.lax.top_k + softmax semantics (stable descending order).

Sharding: data-parallel over tokens across 8 NeuronCores; each core runs an
identical program on its 131072-token slice. Per core, tokens are laid out
partition-major: token(p, i, t) = p*1024 + i*T + t so every DMA touches
contiguous >=1KB runs per partition.

Per 128-token group the DVE executes one InstMax (top-8 values, descending,
exact f32 compare) and one InstMaxIndex (stable first-match indices — ties
resolved to the smallest index, same as jax.lax.top_k). Softmax over the 8
selected logits: exp on ScalarE (no max-subtraction needed; |logit| <= ~6),
segmented sum + reciprocal on DVE, final scale multiply on GPSIMD.
"""

import sys

if "/opt/trn_rl_repo" not in sys.path:
    sys.path.insert(0, "/opt/trn_rl_repo")

import numpy as np

N_TOKENS = 1048576
E = 64            # experts
K = 8             # top-k
NCORES = 8
P = 128           # SBUF partitions
TPC = N_TOKENS // NCORES   # tokens per core = 131072
TPP = TPC // P             # tokens per partition = 1024
T = 64                     # tokens per partition per tile
NT = TPP // T              # tiles per core = 16

_CACHE = {}


def _build(tpp=TPP, t_tile=T):
    import concourse.bacc as bacc
    import concourse.mybir as mybir
    import concourse.tile as tile

    f32 = mybir.dt.float32
    u16 = mybir.dt.uint16

    n_tiles = tpp // t_tile
    n_tok = P * tpp

    nc = bacc.Bacc("TRN2", target_bir_lowering=False, debug=False)
    logits = nc.dram_tensor("logits", [n_tok, E], f32, kind="ExternalInput")
    idx_out = nc.dram_tensor("idx_out", [n_tok, K], u16, kind="ExternalOutput")
    w_out = nc.dram_tensor("w_out", [n_tok, K], f32, kind="ExternalOutput")

    # token(p, i, t) = p*tpp + i*t_tile + t  (partition-major)
    lg_v = logits.ap().rearrange("(p i t) e -> i p t e", p=P, i=n_tiles, t=t_tile)
    ix_v = idx_out.ap().rearrange("(p i t) k -> i p t k", p=P, i=n_tiles, t=t_tile)
    w_v = w_out.ap().rearrange("(p i t) k -> i p t k", p=P, i=n_tiles, t=t_tile)

    with tile.TileContext(nc) as tc:
        with tc.tile_pool(name="io", bufs=4) as pool:
            for i in range(n_tiles):
                x = pool.tile([P, t_tile, E], f32, tag="x")
                nc.sync.dma_start(x[:], lg_v[i])
                vals = pool.tile([P, t_tile, K], f32, tag="vals")
                idx = pool.tile([P, t_tile, K], u16, tag="idx")
                # phase order: all max first, then all max_index — lets the
                # DVE stream back-to-back without RAW stalls inside a pair
                for t in range(t_tile):
                    nc.vector.max(vals[:, t, :], x[:, t, :])
                for t in range(t_tile):
                    nc.vector.max_index(idx[:, t, :], vals[:, t, :], x[:, t, :])
                ex = pool.tile([P, t_tile, K], f32, tag="ex")
                nc.scalar.activation(
                    ex[:], vals[:], mybir.ActivationFunctionType.Exp
                )
                s = pool.tile([P, t_tile, 1], f32, tag="s")
                nc.vector.reduce_sum(s[:], ex[:], axis=mybir.AxisListType.X)
                r = pool.tile([P, t_tile, 1], f32, tag="r")
                nc.vector.reciprocal(r[:], s[:])
                w = pool.tile([P, t_tile, K], f32, tag="w")
                nc.gpsimd.tensor_mul(w[:], ex[:], r[:].broadcast_to([P, t_tile, K]))
                nc.sync.dma_start(ix_v[i], idx[:])
                nc.sync.dma_start(w_v[i], w[:])
    nc.compile()
    return nc


def _get_nc():
    if "nc" not in _CACHE:
        _CACHE["nc"] = _build()
    return _CACHE["nc"]


def kernel(logits, top_k):
    logits = np.asarray(logits, dtype=np.float32)
    k = int(np.asarray(top_k))
    assert k == K, f"kernel hardcodes top_k={K}, got {k}"
    assert logits.shape == (N_TOKENS, E), logits.shape

    from concourse.bass_utils import run_bass_kernel_spmd

    nc = _get_nc()
    chunks = logits.reshape(NCORES, TPC, E)
    in_maps = [{"logits": np.ascontiguousarray(chunks[c])} for c in range(NCORES)]
    res = run_bass_kernel_spmd(nc, in_maps, list(range(NCORES)))

    # device layout -> natural token order: idx_out rows are already in
    # natural order (the DRAM views wrote token p*1024 + i*T + t at row
    # p*1024 + i*T + t), so a plain concat along tokens is correct.
    idx = np.concatenate([r["idx_out"] for r in res.results], axis=0)
    w = np.concatenate([r["w_out"] for r in res.results], axis=0)
    return idx.astype(np.int64), w.astype(np.float32)


# revision 2
# speedup vs baseline: 1.0170x; 1.0170x over previous
"""Trainium2 Bass kernel: MoE top-k router (top-8 of 64 experts + softmax).

Contract: kernel(logits, top_k) takes the FULL inputs (logits [1048576, 64]
f32, top_k == 8) and returns (topk_idx int64 [N, 8], topk_w f32 [N, 8]),
matching jax.lax.top_k + softmax semantics (stable descending order).

Sharding: data-parallel over tokens across 8 NeuronCores; each core runs an
identical program on its 131072-token slice. Per core, tokens are laid out
partition-major: token(p, i, t) = p*1024 + i*T + t so every DMA touches
contiguous >=1KB runs per partition.

Per 128-token group the DVE executes one InstMax (top-8 values, descending,
exact f32 compare) and one InstMaxIndex (stable first-match indices — ties
resolved to the smallest index, same as jax.lax.top_k). Softmax over the 8
selected logits: exp on ScalarE (no max-subtraction needed; |logit| <= ~6),
segmented sum + reciprocal on DVE, final scale multiply on GPSIMD.
"""

import sys

if "/opt/trn_rl_repo" not in sys.path:
    sys.path.insert(0, "/opt/trn_rl_repo")

import numpy as np

N_TOKENS = 1048576
E = 64            # experts
K = 8             # top-k
NCORES = 8
P = 128           # SBUF partitions
TPC = N_TOKENS // NCORES   # tokens per core = 131072
TPP = TPC // P             # tokens per partition = 1024
T = 64                     # tokens per partition per tile
NT = TPP // T              # tiles per core = 16

_CACHE = {}


def _build(tpp=TPP, t_tile=T):
    import concourse.bacc as bacc
    import concourse.mybir as mybir
    import concourse.tile as tile

    f32 = mybir.dt.float32
    u16 = mybir.dt.uint16

    n_tiles = tpp // t_tile
    n_tok = P * tpp

    nc = bacc.Bacc("TRN2", target_bir_lowering=False, debug=False)
    logits = nc.dram_tensor("logits", [n_tok, E], f32, kind="ExternalInput")
    idx_out = nc.dram_tensor("idx_out", [n_tok, K], u16, kind="ExternalOutput")
    w_out = nc.dram_tensor("w_out", [n_tok, K], f32, kind="ExternalOutput")

    # token(p, i, t) = p*tpp + i*t_tile + t  (partition-major)
    lg_v = logits.ap().rearrange("(p i t) e -> i p t e", p=P, i=n_tiles, t=t_tile)
    ix_v = idx_out.ap().rearrange("(p i t) k -> i p t k", p=P, i=n_tiles, t=t_tile)
    w_v = w_out.ap().rearrange("(p i t) k -> i p t k", p=P, i=n_tiles, t=t_tile)

    with tile.TileContext(nc) as tc:
        with tc.tile_pool(name="io", bufs=4) as pool:
            for i in range(n_tiles):
                x = pool.tile([P, t_tile, E], f32, tag="x")
                nc.sync.dma_start(x[:], lg_v[i])
                vals = pool.tile([P, t_tile, K], f32, tag="vals")
                idx = pool.tile([P, t_tile, K], u16, tag="idx")
                # phase order: all max first, then all max_index — lets the
                # DVE stream back-to-back without RAW stalls inside a pair
                for t in range(t_tile):
                    nc.vector.max(vals[:, t, :], x[:, t, :])
                for t in range(t_tile):
                    nc.vector.max_index(idx[:, t, :], vals[:, t, :], x[:, t, :])
                ex = pool.tile([P, t_tile, K], f32, tag="ex")
                nc.scalar.activation(
                    ex[:], vals[:], mybir.ActivationFunctionType.Exp
                )
                # softmax denominator: pairwise tree-sum on GPSIMD (keeps the
                # DVE free for MAX8/FIND_INDEX8, which are its bottleneck)
                t1 = pool.tile([P, t_tile, 4], f32, tag="t1")
                t2 = pool.tile([P, t_tile, 2], f32, tag="t2")
                s = pool.tile([P, t_tile, 1], f32, tag="s")
                nc.gpsimd.tensor_add(t1[:], ex[:, :, 0:4], ex[:, :, 4:8])
                nc.gpsimd.tensor_add(t2[:], t1[:, :, 0:2], t1[:, :, 2:4])
                nc.gpsimd.tensor_add(s[:], t2[:, :, 0:1], t2[:, :, 1:2])
                r = pool.tile([P, t_tile, 1], f32, tag="r")
                nc.vector.reciprocal(r[:], s[:])
                w = pool.tile([P, t_tile, K], f32, tag="w")
                nc.gpsimd.tensor_mul(w[:], ex[:], r[:].broadcast_to([P, t_tile, K]))
                nc.sync.dma_start(ix_v[i], idx[:])
                nc.sync.dma_start(w_v[i], w[:])
    nc.compile()
    return nc


def _get_nc():
    if "nc" not in _CACHE:
        _CACHE["nc"] = _build()
    return _CACHE["nc"]


def kernel(logits, top_k):
    logits = np.asarray(logits, dtype=np.float32)
    k = int(np.asarray(top_k))
    assert k == K, f"kernel hardcodes top_k={K}, got {k}"
    assert logits.shape == (N_TOKENS, E), logits.shape

    from concourse.bass_utils import run_bass_kernel_spmd

    nc = _get_nc()
    chunks = logits.reshape(NCORES, TPC, E)
    in_maps = [{"logits": np.ascontiguousarray(chunks[c])} for c in range(NCORES)]
    res = run_bass_kernel_spmd(nc, in_maps, list(range(NCORES)))

    # device layout -> natural token order: idx_out rows are already in
    # natural order (the DRAM views wrote token p*1024 + i*T + t at row
    # p*1024 + i*T + t), so a plain concat along tokens is correct.
    idx = np.concatenate([r["idx_out"] for r in res.results], axis=0)
    w = np.concatenate([r["w_out"] for r in res.results], axis=0)
    return idx.astype(np.int64), w.astype(np.float32)
